# revision 18
# baseline (speedup 1.0000x reference)
"""ContextQueryAttention (BiDAF-style) Trainium2 kernel, v2.

Shapes (hardcoded): B=32, D=128, C=1024, Q=128, fp32 I/O.
Sharding: data-parallel over batch B across 8 NeuronCores (4 batches/core).

Math per batch (b fixed), with S[i,j] = pc[i] + pq[j] + cq[i,j] (+bias, which
cancels in both softmaxes):
  E2[j,i]  = exp(pq[j] + cq[i,j] - SHIFT)       [Q,C] j-major, 2 wide matmuls
             (lhsT = wqq stationary, rhs = ctx) + exp with per-partition bias
  E2T      = PE-transpose of E2 chunks           [C,Q] i-major
  R[i]     = sum_j E2[j,i]   (row-softmax normalizer, free-axis reduce on E2T)
  u[j,d+1] = sum_i E2T[i,j] * [epc*ctxT | epc][i,d]   (col-softmax via epc fold)
  tT[j,d]  = u[j,0:D] / u[j,D]                   (exact S_col^T @ ctx^T rows)
  finals   = E2 chunk^T @ [qT | tT]  -> [c2qT_u | q2cT_u] per 128-chunk of i
Device ships c2qT_u, q2cT_u (unnormalized) and rr = 1/R; host computes
  c2q[d,i] = c2qT_u[i,d]*rr[i],  q2c[d,i] = q2cT_u[i,d]*rr[i]
  out = stack([ctx, c2q, ctx*c2q, ctx*q2c]).

All matmul operands fp16 (PSUM fp32); pq enters exp as fp32 bias (exact);
pc enters via exp(pc - max pc) folded into ctxT on host (cancels in t).
"""

import os
from contextlib import ExitStack

import numpy as np

import concourse.bacc as bacc
import concourse.tile as tile
from concourse import mybir
from concourse.bass_utils import run_bass_kernel_spmd

B, D, C, Q = 32, 128, 1024, 128
N_CORES = 8
BPC = B // N_CORES  # batches per core
NCH = C // 128      # 8 C-chunks of 128
F32 = mybir.dt.float32
F16 = mybir.dt.float16

TRACE = os.environ.get("CQA_TRACE", "0") == "1"
WARMUP = int(os.environ.get("CQA_WARMUP", "30"))
LAST_EXEC_NS = None
LAST_RESULTS = None

EXP_SHIFT = 6.0  # constant shift inside exp; cancels everywhere downstream

# per-batch column offsets inside each batch's input tile
OFF_WQQ = 0
OFF_CTX = 128
OFF_QT = 128 + 1024          # 1152
OFF_TT = OFF_QT + 128        # 1280 (device-written tT slot; shipped as zeros)
OFF_CTW = OFF_TT + 128       # 1408, ctxTw_aug [8 chunks x 129]
BATW = OFF_CTW + NCH * (D + 1)  # 2440

_compiled = {}


def _build_v2():
    nc = bacc.Bacc(None)
    EXP = mybir.ActivationFunctionType.Exp

    big_d = nc.declare_dram_parameter("bigin", [BPC, 128, BATW], F16, isOutput=False)
    id_d = nc.declare_dram_parameter("identity", [128, 128], F16, isOutput=False)
    smalls_d = nc.declare_dram_parameter("smalls", [128, BPC], F32, isOutput=False)
    out_d = nc.declare_dram_parameter("out", [BPC, 128, 2 * C], F16, isOutput=True)
    rout_d = nc.declare_dram_parameter("rout", [1, BPC * C], F32, isOutput=True)

    with tile.TileContext(nc) as tc, ExitStack() as ctx:
        const = ctx.enter_context(tc.tile_pool(name="const", bufs=1))
        inp = ctx.enter_context(tc.tile_pool(name="inp", bufs=BPC))
        work = ctx.enter_context(tc.tile_pool(name="work", bufs=2))
        outp = ctx.enter_context(tc.tile_pool(name="outp", bufs=2))
        psB = ctx.enter_context(tc.tile_pool(name="psB", bufs=2, space="PSUM"))
        psT = ctx.enter_context(tc.tile_pool(name="psT", bufs=2, space="PSUM"))
        psU = ctx.enter_context(tc.tile_pool(name="psU", bufs=2, space="PSUM"))
        psF = ctx.enter_context(tc.tile_pool(name="psF", bufs=2, space="PSUM"))

        # Input DMAs, critical-first: batch 0 split so MM-B can start early.
        smalls_sb = const.tile([128, BPC], F32, tag="smalls")
        nc.scalar.dma_start(out=smalls_sb[:], in_=smalls_d[:])
        big_sb = []
        for b in range(BPC):
            big_sb.append(
                inp.tile([128, BATW], F16, tag="big", name=f"big{b}")
            )
        nc.sync.dma_start(out=big_sb[0][:, 0:OFF_QT], in_=big_d[0][:, 0:OFF_QT])
        nc.scalar.dma_start(out=big_sb[0][:, OFF_QT:BATW], in_=big_d[0][:, OFF_QT:BATW])
        ident_sb = const.tile([128, 128], F16, tag="ident")
        nc.sync.dma_start(out=ident_sb[:], in_=id_d[:])
        nc.sync.dma_start(out=big_sb[1][:], in_=big_d[1])
        nc.scalar.dma_start(out=big_sb[2][:], in_=big_d[2])
        nc.sync.dma_start(out=big_sb[3][:], in_=big_d[3])

        # PE warmup: dead back-to-back matmuls spanning the startup window
        # (preamble + first input DMA) so the PE clock is ramped when real
        # matmuls begin. Depends only on an on-chip memset.
        rall_sb = const.tile([1, BPC * C], F32, tag="rall")
        wu_sb = const.tile([128, 128], F16, tag="wu")
        nc.gpsimd.memset(wu_sb[:], 0.0)
        wu_ps = psF.tile([128, 512], F32, tag="F")
        wu_sink = const.tile([128, 1], F32, tag="wu_sink")
        for _ in range(WARMUP):
            nc.tensor.matmul(
                out=wu_ps[:, 0:128],
                lhsT=wu_sb[:],
                rhs=wu_sb[:],
                start=True,
                stop=True,
            )
        nc.scalar.copy(out=wu_sink[:], in_=wu_ps[:, 0:1])

        for b in range(BPC):
            bb = big_sb[b]
            wqq_v = bb[:, OFF_WQQ : OFF_WQQ + 128]
            ctx_v = bb[:, OFF_CTX : OFF_CTX + C]
            rhs_cat = bb[:, OFF_QT : OFF_QT + 256]  # [qT | tT(slot)]
            tt_v = bb[:, OFF_TT : OFF_TT + 128]
            ctw_v = bb[:, OFF_CTW : OFF_CTW + NCH * (D + 1)].rearrange(
                "p (c m) -> p c m", m=D + 1
            )

            E2_sb = work.tile([128, C], F16, tag="E2")
            E2T_sb = work.tile([128, C], F16, tag="E2T")
            r_sb = work.tile([Q, 1], F32, tag="r")
            out_sb = outp.tile([128, 2 * C], F16, tag="out")

            # E2 = exp(cq^T + pq - SHIFT), j-major, one stationary weight.
            for h in range(2):
                ps = psB.tile([128, 512], F32, tag="S")
                nc.tensor.matmul(
                    out=ps[:],
                    lhsT=wqq_v,
                    rhs=ctx_v[:, h * 512 : (h + 1) * 512],
                    start=True,
                    stop=True,
                )
                nc.scalar.activation(
                    out=E2_sb[:, h * 512 : (h + 1) * 512],
                    in_=ps[:],
                    func=EXP,
                    bias=smalls_sb[:, b : b + 1],
                )
                # transpose 4 chunks of this half into PSUM
                pt = psT.tile([128, 512], F16, tag="T")
                for k in range(4):
                    c = h * 4 + k
                    nc.tensor.transpose(
                        out=pt[:, k * 128 : (k + 1) * 128],
                        in_=E2_sb[:, c * 128 : (c + 1) * 128],
                        identity=ident_sb[:],
                    )
                if h == 0:
                    nc.scalar.copy(
                        out=E2T_sb[:, h * 512 : (h + 1) * 512], in_=pt[:]
                    )
                else:
                    nc.vector.tensor_copy(
                        E2T_sb[:, h * 512 : (h + 1) * 512], pt[:]
                    )
            # R[i] = sum_j E2[j,i]: partition-axis reduce, shipped raw to host
            nc.gpsimd.tensor_reduce(
                out=rall_sb[0:1, b * C : (b + 1) * C],
                in_=E2_sb[:],
                axis=mybir.AxisListType.C,
                op=mybir.AluOpType.add,
            )

            # u accumulation over C chunks; col D is V[j] = sum_i E2T*epc.
            psu = psU.tile([Q, D + 1], F32, tag="U")
            for c in range(NCH):
                nc.tensor.matmul(
                    out=psu[:],
                    lhsT=E2T_sb[:, c * 128 : (c + 1) * 128],
                    rhs=ctw_v[:, c, :],
                    start=(c == 0),
                    stop=(c == NCH - 1),
                )
            nc.vector.reciprocal(out=r_sb[:], in_=psu[:, D : D + 1])
            nc.vector.tensor_scalar_mul(tt_v, psu[:, 0:D], r_sb[:])

            # finals: per chunk c, E2c^T @ [qT | tT] -> [c2qT_u | q2cT_u]
            for hh in range(4):
                pf = psF.tile([128, 512], F32, tag="F")
                for k in range(2):
                    c = 2 * hh + k
                    nc.tensor.matmul(
                        out=pf[:, k * 256 : (k + 1) * 256],
                        lhsT=E2_sb[:, c * 128 : (c + 1) * 128],
                        rhs=rhs_cat,
                        start=True,
                        stop=True,
                    )
                if hh % 2 == 0:
                    nc.scalar.copy(
                        out=out_sb[:, hh * 512 : (hh + 1) * 512], in_=pf[:]
                    )
                else:
                    nc.vector.tensor_copy(
                        out_sb[:, hh * 512 : (hh + 1) * 512], pf[:]
                    )
            if b == BPC - 1:
                nc.sync.dma_start(out=out_d[b][:, 0:C], in_=out_sb[:, 0:C])
                nc.gpsimd.dma_start(
                    out=out_d[b][:, C : 2 * C], in_=out_sb[:, C : 2 * C]
                )
            elif b % 2 == 0:
                nc.sync.dma_start(out=out_d[b][:], in_=out_sb[:])
            else:
                nc.gpsimd.dma_start(out=out_d[b][:], in_=out_sb[:])

        nc.sync.dma_start(out=rout_d[:], in_=rall_sb[:])

    nc.finalize()
    return nc


def kernel(context, question, w_c, w_q, w_cq, bias):
    global LAST_EXEC_NS, LAST_RESULTS
    ctx = np.ascontiguousarray(np.asarray(context, dtype=np.float32))
    qst = np.ascontiguousarray(np.asarray(question, dtype=np.float32))
    w_c = np.asarray(w_c, dtype=np.float32)
    w_q = np.asarray(w_q, dtype=np.float32)
    w_cq = np.asarray(w_cq, dtype=np.float32)
    # bias is an additive constant inside both softmaxes and cancels; unused.

    if "v2" not in _compiled:
        _compiled["v2"] = _build_v2()
    nc = _compiled["v2"]

    wq_q = (w_cq[None, :, None] * qst).astype(np.float32)          # [B, D, Q]
    part_q = np.einsum("d,bdj->bj", w_q, qst).astype(np.float32)   # [B, Q]
    part_c = np.einsum("d,bdi->bi", w_c, ctx).astype(np.float32)   # [B, C]
    ctxT = ctx.transpose(0, 2, 1)                                  # [B, C, D]

    # epc normalized per batch so f16 stays well-conditioned; cancels in t.
    epc = np.exp(part_c - part_c.max(axis=1, keepdims=True))       # [B, C]
    ctw = np.concatenate(
        [ctxT * epc[:, :, None], epc[:, :, None]], axis=2
    ).astype(np.float16)                                           # [B, C, D+1]
    ctw_pm = (
        ctw.reshape(B, NCH, 128, D + 1)
        .transpose(0, 2, 1, 3)
        .reshape(B, 128, NCH * (D + 1))
    )

    big = np.zeros((B, 128, BATW), np.float16)
    big[:, :, OFF_WQQ : OFF_WQQ + 128] = wq_q
    big[:, :, OFF_CTX : OFF_CTX + C] = ctx
    big[:, :, OFF_QT : OFF_QT + 128] = qst.transpose(0, 2, 1)
    big[:, :, OFF_CTW : OFF_CTW + NCH * (D + 1)] = ctw_pm

    smalls = np.ascontiguousarray(
        (part_q - EXP_SHIFT).reshape(N_CORES, BPC, 128).transpose(0, 2, 1)
    ).astype(np.float32)                                           # [8, 128, BPC]

    identity = np.eye(128, dtype=np.float16)
    in_maps = []
    for i in range(N_CORES):
        s = slice(i * BPC, (i + 1) * BPC)
        in_maps.append(
            {
                "bigin": np.ascontiguousarray(big[s]),
                "identity": identity,
                "smalls": smalls[i],
            }
        )

    res = run_bass_kernel_spmd(
        nc, in_maps, core_ids=list(range(N_CORES)), trace=TRACE
    )
    LAST_EXEC_NS = res.exec_time_ns
    LAST_RESULTS = res

    out = np.empty((4, B, D, C), dtype=np.float32)
    out[0] = ctx
    for i in range(N_CORES):
        dev = res.results[i]["out"].astype(np.float32)  # [BPC, 128, 2C]
        rout = res.results[i]["rout"].reshape(BPC, C)   # f32
        for bb in range(BPC):
            bg = i * BPC + bb
            o = dev[bb].reshape(128, NCH, 2, 128)
            rr = (1.0 / rout[bb]).reshape(NCH, 128).T   # [128(p), NCH]
            # c2qT_u[c*128+p, d] = o[p, c, 0, d]; scale by 1/R then transpose
            c2qT = o[:, :, 0, :] * rr[:, :, None]       # [128, NCH, D]
            q2cT = o[:, :, 1, :] * rr[:, :, None]
            out[1, bg] = c2qT.transpose(2, 1, 0).reshape(D, C)
            out[3, bg] = ctx[bg] * q2cT.transpose(2, 1, 0).reshape(D, C)
    out[2] = ctx * out[1]
    return out


# revision 19
# speedup vs baseline: 17.7812x; 17.7812x over previous
"""ContextQueryAttention (BiDAF-style) Trainium2 kernel, v2.

Shapes (hardcoded): B=32, D=128, C=1024, Q=128, fp32 I/O.
Sharding: data-parallel over batch B across 8 NeuronCores (4 batches/core).

Math per batch (b fixed), with S[i,j] = pc[i] + pq[j] + cq[i,j] (+bias, which
cancels in both softmaxes):
  E0[i,j]  = exp(cq[i,j])                [C,Q] i-major chunks (t path)
  E2[j,i]  = exp(pq[j] + cq[i,j] - 6)    [Q,C] j-major, 2 wide matmuls with
             wqq stationary + exp with per-partition fp32 bias
  u[j,d+1] = sum_i E0[i,j] * [epc*ctxT | epc][i,d]   (epc host-folded)
  tT[j,d]  = u[j,0:D] / u[j,D]           (= rows of S_col^T @ ctx^T, exact)
  finals   = E2c^T @ [qT | tT | 1] -> [c2qT_u | q2cT_u | R] per 128-chunk c
Device ships c2qT_u, q2cT_u, R (unnormalized); host computes
  c2q[d,i] = c2qT_u[i,d]/R[i],  q2c[d,i] = q2cT_u[i,d]/R[i]
  out = stack([ctx, c2q, ctx*c2q, ctx*q2c]).

All matmul operands fp16 (PSUM accumulation fp32); pq enters exp as fp32 bias
(exact); pc enters via epc = exp(pc - max pc) folded into ctxT on host, which
cancels in the t ratio. Shifts cancel identically in all normalized outputs.
"""

import os
from contextlib import ExitStack

import numpy as np

import concourse.bacc as bacc
import concourse.tile as tile
from concourse import mybir
from concourse.bass_utils import run_bass_kernel_spmd

B, D, C, Q = 32, 128, 1024, 128
N_CORES = 8
BPC = B // N_CORES  # batches per core
NCH = C // 128      # 8 C-chunks of 128
F32 = mybir.dt.float32
F16 = mybir.dt.float16

TRACE = os.environ.get("CQA_TRACE", "0") == "1"
WARMUP = int(os.environ.get("CQA_WARMUP", "30"))
LAST_EXEC_NS = None
LAST_RESULTS = None

EXP_SHIFT = 6.0  # constant shift inside E2's exp; cancels downstream

# per-batch column offsets inside each batch's input tile
OFF_WQQ = 0
OFF_CTX = 128
OFF_QT = 128 + 1024          # 1152
OFF_TT = OFF_QT + 128        # 1280 (device-written tT slot; shipped as zeros)
OFF_CTW = OFF_TT + 128       # 1408, ctxTw_aug [8 chunks x 129]
BATW = OFF_CTW + NCH * (D + 1)  # 2440

OW = NCH * 257  # 2056: per chunk [c2qT_u(128) | q2cT_u(128) | R(1)]

_compiled = {}


def _build_v2():
    nc = bacc.Bacc(None)
    EXP = mybir.ActivationFunctionType.Exp

    big_d = nc.declare_dram_parameter("bigin", [BPC, 128, BATW], F16, isOutput=False)
    smalls_d = nc.declare_dram_parameter("smalls", [128, BPC], F32, isOutput=False)
    out_d = nc.declare_dram_parameter("out", [BPC, 128, OW], F16, isOutput=True)

    with tile.TileContext(nc) as tc, ExitStack() as ctx:
        const = ctx.enter_context(tc.tile_pool(name="const", bufs=1))
        inp = ctx.enter_context(tc.tile_pool(name="inp", bufs=BPC))
        work = ctx.enter_context(tc.tile_pool(name="work", bufs=2))
        outp = ctx.enter_context(tc.tile_pool(name="outp", bufs=2))
        psSA = ctx.enter_context(tc.tile_pool(name="psSA", bufs=2, space="PSUM"))
        psU = ctx.enter_context(tc.tile_pool(name="psU", bufs=2, space="PSUM"))
        psF = ctx.enter_context(tc.tile_pool(name="psF", bufs=4, space="PSUM"))

        # Input DMAs, critical-first: batch 0 split so compute starts early.
        smalls_sb = const.tile([128, BPC], F32, tag="smalls")
        nc.scalar.dma_start(out=smalls_sb[:], in_=smalls_d[:])
        big_sb = []
        for b in range(BPC):
            big_sb.append(
                inp.tile([128, BATW], F16, tag="big", name=f"big{b}")
            )
        nc.sync.dma_start(out=big_sb[0][:, 0:OFF_QT], in_=big_d[0][:, 0:OFF_QT])
        nc.scalar.dma_start(
            out=big_sb[0][:, OFF_QT:BATW], in_=big_d[0][:, OFF_QT:BATW]
        )
        nc.sync.dma_start(out=big_sb[1][:], in_=big_d[1])
        nc.scalar.dma_start(out=big_sb[2][:], in_=big_d[2])
        nc.sync.dma_start(out=big_sb[3][:], in_=big_d[3])

        # PE warmup: dead back-to-back matmuls spanning the startup window
        # (preamble + first input DMA) so the PE clock is ramped when real
        # matmuls begin. Depends only on an on-chip memset.
        wu_sb = const.tile([128, 128], F16, tag="wu")
        nc.gpsimd.memset(wu_sb[:], 0.0)
        ones_sb = const.tile([128, 1], F16, tag="ones")
        nc.gpsimd.memset(ones_sb[:], 1.0)
        wu_ps = psF.tile([128, 257], F32, tag="F")
        wu_sink = const.tile([128, 1], F32, tag="wu_sink")
        for _ in range(WARMUP):
            nc.tensor.matmul(
                out=wu_ps[:, 0:128],
                lhsT=wu_sb[:],
                rhs=wu_sb[:],
                start=True,
                stop=True,
            )
        nc.scalar.copy(out=wu_sink[:], in_=wu_ps[:, 0:1])

        for b in range(BPC):
            bb = big_sb[b]
            wqq_v = bb[:, OFF_WQQ : OFF_WQQ + 128]
            ctx_v = bb[:, OFF_CTX : OFF_CTX + C]
            rhs_cat = bb[:, OFF_QT : OFF_QT + 256]  # [qT | tT(slot)]
            tt_v = bb[:, OFF_TT : OFF_TT + 128]
            ctw_v = bb[:, OFF_CTW : OFF_CTW + NCH * (D + 1)].rearrange(
                "p (c m) -> p c m", m=D + 1
            )

            E0_sb = work.tile([128, C], F16, tag="E0")
            E2_sb = work.tile([128, C], F16, tag="E2")
            r_sb = work.tile([Q, 1], F32, tag="r")
            out_sb = outp.tile([128, OW], F16, tag="out")

            # E0 = exp(cq), i-major chunks (for the column softmax / t path)
            for h in range(2):
                ps = psSA.tile([128, 512], F32, tag="S")
                for k in range(4):
                    c = h * 4 + k
                    nc.tensor.matmul(
                        out=ps[:, k * 128 : (k + 1) * 128],
                        lhsT=ctx_v[:, c * 128 : (c + 1) * 128],
                        rhs=wqq_v,
                        start=True,
                        stop=True,
                    )
                nc.scalar.activation(
                    out=E0_sb[:, h * 512 : (h + 1) * 512],
                    in_=ps[:],
                    func=EXP,
                )

            # E2 = exp(cq^T + pq - SHIFT), j-major, one stationary weight
            for h in range(2):
                ps = psSA.tile([128, 512], F32, tag="S")
                nc.tensor.matmul(
                    out=ps[:],
                    lhsT=wqq_v,
                    rhs=ctx_v[:, h * 512 : (h + 1) * 512],
                    start=True,
                    stop=True,
                )
                nc.scalar.activation(
                    out=E2_sb[:, h * 512 : (h + 1) * 512],
                    in_=ps[:],
                    func=EXP,
                    bias=smalls_sb[:, b : b + 1],
                )

            # u accumulation over C chunks; col D is V[j] = sum_i E0*epc.
            psu = psU.tile([Q, D + 1], F32, tag="U")
            for c in range(NCH):
                nc.tensor.matmul(
                    out=psu[:],
                    lhsT=E0_sb[:, c * 128 : (c + 1) * 128],
                    rhs=ctw_v[:, c, :],
                    start=(c == 0),
                    stop=(c == NCH - 1),
                )
            nc.vector.reciprocal(out=r_sb[:], in_=psu[:, D : D + 1])
            nc.vector.tensor_scalar_mul(tt_v, psu[:, 0:D], r_sb[:])

            # finals: per chunk c, E2c^T @ [qT | tT | 1] -> [c2qT_u|q2cT_u|R]
            for c in range(NCH):
                pf = psF.tile([128, 257], F32, tag="F", name=f"pf{c}")
                nc.tensor.matmul(
                    out=pf[:, 0:256],
                    lhsT=E2_sb[:, c * 128 : (c + 1) * 128],
                    rhs=rhs_cat,
                    start=True,
                    stop=True,
                )
                nc.tensor.matmul(
                    out=pf[:, 256:257],
                    lhsT=E2_sb[:, c * 128 : (c + 1) * 128],
                    rhs=ones_sb[:],
                    start=True,
                    stop=True,
                )
                if c % 2 == 0:
                    nc.scalar.copy(
                        out=out_sb[:, c * 257 : (c + 1) * 257], in_=pf[:]
                    )
                else:
                    nc.vector.tensor_copy(
                        out_sb[:, c * 257 : (c + 1) * 257], pf[:]
                    )

            if b == BPC - 1:
                nc.sync.dma_start(out=out_d[b][:, 0:1028], in_=out_sb[:, 0:1028])
                nc.gpsimd.dma_start(
                    out=out_d[b][:, 1028:OW], in_=out_sb[:, 1028:OW]
                )
            elif b % 2 == 0:
                nc.sync.dma_start(out=out_d[b][:], in_=out_sb[:])
            else:
                nc.gpsimd.dma_start(out=out_d[b][:], in_=out_sb[:])

    nc.finalize()
    return nc


def kernel(context, question, w_c, w_q, w_cq, bias):
    global LAST_EXEC_NS, LAST_RESULTS
    ctx = np.ascontiguousarray(np.asarray(context, dtype=np.float32))
    qst = np.ascontiguousarray(np.asarray(question, dtype=np.float32))
    w_c = np.asarray(w_c, dtype=np.float32)
    w_q = np.asarray(w_q, dtype=np.float32)
    w_cq = np.asarray(w_cq, dtype=np.float32)
    # bias is an additive constant inside both softmaxes and cancels; unused.

    if "v2" not in _compiled:
        _compiled["v2"] = _build_v2()
    nc = _compiled["v2"]

    wq_q = (w_cq[None, :, None] * qst).astype(np.float32)          # [B, D, Q]
    part_q = np.einsum("d,bdj->bj", w_q, qst).astype(np.float32)   # [B, Q]
    part_c = np.einsum("d,bdi->bi", w_c, ctx).astype(np.float32)   # [B, C]
    ctxT = ctx.transpose(0, 2, 1)                                  # [B, C, D]

    # epc normalized per batch so f16 stays well-conditioned; cancels in t.
    epc = np.exp(part_c - part_c.max(axis=1, keepdims=True))       # [B, C]
    ctw = np.concatenate(
        [ctxT * epc[:, :, None], epc[:, :, None]], axis=2
    ).astype(np.float16)                                           # [B, C, D+1]
    ctw_pm = (
        ctw.reshape(B, NCH, 128, D + 1)
        .transpose(0, 2, 1, 3)
        .reshape(B, 128, NCH * (D + 1))
    )

    big = np.zeros((B, 128, BATW), np.float16)
    big[:, :, OFF_WQQ : OFF_WQQ + 128] = wq_q
    big[:, :, OFF_CTX : OFF_CTX + C] = ctx
    big[:, :, OFF_QT : OFF_QT + 128] = qst.transpose(0, 2, 1)
    big[:, :, OFF_CTW : OFF_CTW + NCH * (D + 1)] = ctw_pm

    smalls = np.ascontiguousarray(
        (part_q - EXP_SHIFT).reshape(N_CORES, BPC, 128).transpose(0, 2, 1)
    ).astype(np.float32)                                           # [8, 128, BPC]

    in_maps = []
    for i in range(N_CORES):
        s = slice(i * BPC, (i + 1) * BPC)
        in_maps.append(
            {
                "bigin": np.ascontiguousarray(big[s]),
                "smalls": smalls[i],
            }
        )

    res = run_bass_kernel_spmd(
        nc, in_maps, core_ids=list(range(N_CORES)), trace=TRACE
    )
    LAST_EXEC_NS = res.exec_time_ns
    LAST_RESULTS = res

    out = np.empty((4, B, D, C), dtype=np.float32)
    out[0] = ctx
    for i in range(N_CORES):
        dev = res.results[i]["out"].astype(np.float32)  # [BPC, 128, OW]
        for bb in range(BPC):
            bg = i * BPC + bb
            o = dev[bb].reshape(128, NCH, 257)
            rr = 1.0 / o[:, :, 256]                     # [128(p), NCH]
            # c2qT_u[c*128+p, d] = o[p, c, d]; scale by 1/R then transpose
            c2qT = o[:, :, 0:128] * rr[:, :, None]      # [128, NCH, D]
            q2cT = o[:, :, 128:256] * rr[:, :, None]
            out[1, bg] = c2qT.transpose(2, 1, 0).reshape(D, C)
            out[3, bg] = ctx[bg] * q2cT.transpose(2, 1, 0).reshape(D, C)
    out[2] = ctx * out[1]
    return out
